# revision 2
# baseline (speedup 1.0000x reference)
"""MoE transformer block on 8 TRN2 NeuronCores.

Sharding: data-parallel over batch (4 batches = 784 tokens per core), no
collectives.  On-chip layout is feature-major ([d, tokens]) for everything
except vh (token-major, needed as ctx-matmul lhsT).  Matmuls run in float32r
(1 cyc/row at free-dim >= 256); fp32r operands are produced either by
gpsimd cast-DMA (weights) or by engine evictions writing float32r tiles.

PSUM discipline: two pools, one unified tag each (every psum tile <= 1 bank,
4 bufs per pool -> exactly 8 banks).  The MoE y-phase holds 3+3 accumulators
across the K(=F) loop while the next expert's h-phase double-buffers 1+1.
"""
import sys

sys.path.insert(0, "/opt/trn_rl_repo")

from contextlib import ExitStack

import numpy as np

import concourse.bass as bass
import concourse.tile as tile
from concourse import bacc, mybir
from concourse.bass_utils import run_bass_kernel_spmd
from concourse.masks import make_identity

FP32 = mybir.dt.float32
FP32R = mybir.dt.float32r
BF16 = mybir.dt.bfloat16
FP8 = mybir.dt.float8e4
DR = mybir.MatmulPerfMode.DoubleRow
AF = mybir.ActivationFunctionType
OP = mybir.AluOpType

# fp8 quantization scales (powers of 2; descales folded into free spots)
S_X = 8.0     # x -> x8, descaled via 1/(S_X*S_W1) in the gelu activation
S_W1 = 16.0
S_W2 = 16.0   # descaled via sel8 = 1/S_W2 (gate broadcast)
S_A = 8.0     # q8/k8; descaled at the qh/kh evictions together with S_W
S_W = 16.0    # Wq8/Wk8/Wo8
S_AV = 4.0    # v8; S_AV*S_WV rides through vh/ctx into cx8, descaled at r1
S_WV = 8.0    # Wv8
BSP = 208     # v8 per-batch stride (196 padded to 16B multiple)

B, S, D, H, E, F = 32, 196, 768, 12, 8, 3072
DH = D // H                 # 64
NCORES = 8
BPC = B // NCORES           # 4 batches per core
T = BPC * S                 # 784 tokens per core
TH = T // 2                 # 392 tokens per half (2 batches)
DK = D // 128               # 6
FK = F // 128               # 24
EPS = 1e-5
TT = [(i * 128, 128) for i in range(6)] + [(768, 16)]   # token tiles
BK = [(0, 128), (128, 68)]                              # ki chunks per batch
NCH = ((0, TH), (TH, TH))                               # token halves
THP = 400                                               # fp8 half stride (16B-aligned)
TP = 2 * THP
NCH8 = ((0, TH), (THP, TH))                             # fp8-tile halves
OTT = [(0, 128), (128, 128), (256, 128), (384, 8)]      # out tiles per half

_CACHE = {}


def _build(skip_attn=False, skip_moe=False, repeat=1):
    nc = bacc.Bacc("TRN2", target_bir_lowering=False, debug=False,
                   num_devices=NCORES)

    q8_d = nc.dram_tensor("q8", [128, DK, TP], FP8, kind="ExternalInput").ap()
    k8_d = nc.dram_tensor("k8", [128, DK, TP], FP8, kind="ExternalInput").ap()
    v8_d = nc.dram_tensor("v8", [128, DK, 4 * BSP], FP8,
                          kind="ExternalInput").ap()
    q32_d = nc.dram_tensor("q32", [D, T], FP32, kind="ExternalInput").ap()
    wq8_d = nc.dram_tensor("Wq8", [128, DK, D], FP8, kind="ExternalInput").ap()
    wk8_d = nc.dram_tensor("Wk8", [128, DK, D], FP8, kind="ExternalInput").ap()
    wv8_d = nc.dram_tensor("Wv8", [128, DK, D], FP8, kind="ExternalInput").ap()
    wo8_d = nc.dram_tensor("Wo8", [128, DK, D], FP8, kind="ExternalInput").ap()
    bq_d = nc.dram_tensor("bq", [D], FP32, kind="ExternalInput").ap()
    bk_d = nc.dram_tensor("bk", [D], FP32, kind="ExternalInput").ap()
    bv_d = nc.dram_tensor("bv", [D], FP32, kind="ExternalInput").ap()
    bo_d = nc.dram_tensor("bo", [D], FP32, kind="ExternalInput").ap()
    l1g_d = nc.dram_tensor("ln1_g", [D], FP32, kind="ExternalInput").ap()
    l1b_d = nc.dram_tensor("ln1_b", [D], FP32, kind="ExternalInput").ap()
    l2g_d = nc.dram_tensor("ln2_g", [D], FP32, kind="ExternalInput").ap()
    l2b_d = nc.dram_tensor("ln2_b", [D], FP32, kind="ExternalInput").ap()
    wg_d = nc.dram_tensor("Wg", [D, E], FP32, kind="ExternalInput").ap()
    bg_d = nc.dram_tensor("bg", [E], FP32, kind="ExternalInput").ap()
    w1_d = nc.dram_tensor("W1q", [E, 128, DK, F], FP8,
                          kind="ExternalInput").ap()
    b1_d = nc.dram_tensor("b1", [E, F], FP32, kind="ExternalInput").ap()
    w2_d = nc.dram_tensor("W2q", [E, 128, F // 256, 2, D], FP8,
                          kind="ExternalInput").ap()
    b2_d = nc.dram_tensor("b2", [E, D], FP32, kind="ExternalInput").ap()
    sel_d = nc.dram_tensor("sel8", [E, E * 128], FP32,
                           kind="ExternalInput").ap()
    aux1_d = nc.dram_tensor("aux_ones", [128, 128], FP32,
                            kind="ExternalInput").ap()
    aux64_d = nc.dram_tensor("aux_ones64", [65, 128], FP32,
                             kind="ExternalInput").ap()
    auxe_d = nc.dram_tensor("aux_eps", [1, 1], FP32,
                            kind="ExternalInput").ap()
    out_d = nc.dram_tensor("out", [T, D], FP32, kind="ExternalOutput").ap()

    with tile.TileContext(nc) as tc, ExitStack() as top:
        const = top.enter_context(tc.tile_pool(name="const", bufs=1))
        vecs = top.enter_context(tc.tile_pool(name="vecs", bufs=1))
        rows = top.enter_context(tc.tile_pool(name="rows", bufs=2))
        psA = top.enter_context(tc.tile_pool(name="psA", bufs=4, space="PSUM"))
        psB = top.enter_context(tc.tile_pool(name="psB", bufs=4, space="PSUM"))
        tmp = top.enter_context(tc.tile_pool(name="tmp", bufs=2))
        persist = top.enter_context(tc.tile_pool(name="persist", bufs=1))

        def pa(p, f):
            return psA.tile([p, f], FP32, tag="a", name="pa")

        def pb(p, f):
            return psB.tile([p, f], FP32, tag="b", name="pb")

        # ---------------- constants ----------------
        ident = const.tile([128, 128], FP32, tag="ident")
        make_identity(nc, ident)
        ones_col_r = const.tile([128, 1], FP32R, tag="ones_col_r")
        nc.gpsimd.dma_start(out=ones_col_r[:], in_=aux1_d[:, 0:1])
        ones_row_r = const.tile([1, 128], FP32R, tag="ones_row_r")
        nc.gpsimd.dma_start(out=ones_row_r[:], in_=aux1_d[0:1, :])
        ones_row8_r = const.tile([1, 8], FP32R, tag="ones_row8_r")
        nc.gpsimd.dma_start(out=ones_row8_r[:], in_=aux1_d[0:1, 0:8])
        ones8_col = const.tile([8, 1], FP32, tag="ones8_col")
        nc.sync.dma_start(out=ones8_col[:], in_=aux1_d[0:8, 0:1])
        # row 64 all-ones: lhsT for the 1/s broadcast (base matches pctx[64])
        ones64r = const.tile([65, 128], FP32R, tag="ones64r")
        nc.gpsimd.dma_start(out=ones64r[:], in_=aux64_d[:, :])
        eps_t = const.tile([1, 1], FP32, tag="eps")
        nc.sync.dma_start(out=eps_t[:], in_=auxe_d[:, :])
        # per-expert selector: sel8[i, e*128 + p] = (i == e), host-built
        sel8 = const.tile([8, E * 128], FP32R, tag="sel8")
        nc.gpsimd.dma_start(out=sel8[:], in_=sel_d[:, :])

        def load_col(dvec, nb, dtype=FP32, tag=None):
            # [nb*128] DRAM vector -> [128, nb] feature-major column tile
            raw = rows.tile([nb, 128], FP32, tag="rawvec")
            nc.sync.dma_start(out=raw[:],
                              in_=dvec.rearrange("(a b) -> a b", b=128))
            ps = pb(128, nb)
            nc.tensor.transpose(ps[:], raw[:], ident[:nb, :nb])
            col = vecs.tile([128, nb], dtype, tag=tag)
            nc.vector.tensor_copy(col[:], ps[:])
            return col

        bq_col = load_col(bq_d, DK, tag="bq")
        bk_col = load_col(bk_d, DK, tag="bk")
        bo_col = load_col(bo_d, DK, tag="bo")
        l1g_col = load_col(l1g_d, DK, tag="l1g")
        l1b_col = load_col(l1b_d, DK, tag="l1b")
        l2g_col = load_col(l2g_d, DK, tag="l2g")
        l2b_col = load_col(l2b_d, DK, tag="l2b")
        # S_X-scaled LN1 affine for the fp8 eviction of x
        l1g8_col = vecs.tile([128, DK], FP32, tag="l1g8")
        nc.vector.tensor_scalar_mul(l1g8_col[:], l1g_col[:], S_X)
        l1b8_col = vecs.tile([128, DK], FP32, tag="l1b8")
        nc.vector.tensor_scalar_mul(l1b8_col[:], l1b_col[:], S_X)
        bg_col = vecs.tile([8, 1], FP32, tag="bg")
        nc.sync.dma_start(out=bg_col[:],
                          in_=bg_d.rearrange("(a b) -> a b", b=1))
        wgs = vecs.tile([128, DK, E], FP32R, tag="wg")
        nc.gpsimd.dma_start(
            out=wgs[:], in_=wg_d.rearrange("(kb p) e -> p kb e", p=128))
        b2s = vecs.tile([E, D], FP32R, tag="b2")
        nc.gpsimd.dma_start(out=b2s[:], in_=b2_d[:, :])

        # persistent activations (full T)
        x_t = [persist.tile([128, T], FP32R, tag=f"xt{k}", name=f"xt{k}") for k in range(DK)]
        x8 = persist.tile([128, DK, TP], FP8, tag="x8", name="x8")
        gexp = persist.tile([8, T], FP32, tag="gexp")
        gate = persist.tile([8, T], FP32R, tag="gate")

        def layer_norm(r_tiles, g_col, b_col, out_tiles, out_off, nch_list,
                       out8=None, g8_col=None, b8_col=None):
            # feature-major LN over D=768 partitions (6 tiles); r_tiles fp32r
            for (n0, nl) in nch_list:
                ps_s = pa(1, TH)
                ps_s2 = pa(1, TH)
                sqs = []
                for k in range(DK):
                    sq = tmp.tile([128, TH], FP32R, tag="ln_sq", bufs=6)
                    nc.scalar.activation(sq[:], r_tiles[k][:, n0:n0 + nl],
                                         AF.Square)
                    sqs.append(sq)
                for k in range(DK):
                    nc.tensor.matmul(ps_s[:], ones_col_r[:],
                                     r_tiles[k][:, n0:n0 + nl],
                                     start=(k == 0), stop=(k == DK - 1))
                for k in range(DK):
                    nc.tensor.matmul(ps_s2[:], ones_col_r[:], sqs[k][:],
                                     start=(k == 0), stop=(k == DK - 1))
                m = rows.tile([1, TH], FP32, tag="ln_m", bufs=1)
                m2 = rows.tile([1, TH], FP32, tag="ln_m2", bufs=1)
                nc.vector.tensor_scalar_mul(m[:], ps_s[:], 1.0 / D)
                nc.vector.tensor_scalar_mul(m2[:], ps_s2[:], 1.0 / D)
                mm_ = rows.tile([1, TH], FP32, tag="ln_mm", bufs=1)
                nc.vector.tensor_mul(mm_[:], m[:], m[:])
                var = rows.tile([1, TH], FP32, tag="ln_var", bufs=1)
                nc.vector.tensor_sub(var[:], m2[:], mm_[:])
                sd = rows.tile([1, TH], FP32, tag="ln_sd", bufs=1)
                nc.scalar.activation(sd[:], var[:], AF.Sqrt, bias=eps_t[:])
                rstd = rows.tile([1, TH], FP32R, tag="ln_rstd", bufs=1)
                with nc.allow_low_precision(reason="fp32r matmul operand"):
                    nc.vector.reciprocal(rstd[:], sd[:])
                mr = rows.tile([1, TH], FP32R, tag="ln_mr", bufs=1)
                nc.vector.tensor_mul(mr[:], m[:], rstd[:])
                pR = pb(128, TH)
                nc.tensor.matmul(pR[:], ones_row_r[:], rstd[:],
                                 start=True, stop=True)
                pM = pb(128, TH)
                nc.tensor.matmul(pM[:], ones_row_r[:], mr[:],
                                 start=True, stop=True)
                for k in range(DK):
                    t1 = tmp.tile([128, TH], FP32, tag="ln_t1")
                    nc.vector.tensor_mul(t1[:], r_tiles[k][:, n0:n0 + nl],
                                         pR[:])
                    t2 = t1
                    nc.vector.tensor_sub(t2[:], t1[:], pM[:])
                    o0 = out_off + n0
                    nc.scalar.activation(out_tiles[k][:, o0:o0 + nl],
                                         t2[:], AF.Identity,
                                         bias=b_col[:, k:k + 1],
                                         scale=g_col[:, k:k + 1])
                    if out8 is not None:
                        o8 = (o0 // TH) * THP + (o0 % TH)
                        with nc.allow_low_precision(reason="fp8 moe rhs"):
                            nc.scalar.activation(out8[:, k, o8:o8 + nl],
                                                 t2[:], AF.Identity,
                                                 bias=b8_col[:, k:k + 1],
                                                 scale=g8_col[:, k:k + 1])

        for rep_i in range(repeat):
            # ================= attention (single scope, fp8 proj) =========
            if skip_attn:
                for k in range(DK):
                    nc.gpsimd.dma_start(out=x_t[k][:],
                                        in_=q32_d[k * 128:(k + 1) * 128, :])
                for k in range(DK):
                    for hi, (n0, nl) in enumerate(NCH):
                        with nc.allow_low_precision(reason="fp8 dbg"):
                            nc.vector.tensor_scalar_mul(
                                x8[:, k, hi * THP:hi * THP + nl],
                                x_t[k][:, n0:n0 + nl], S_X)
            if not skip_attn:
                with ExitStack() as hs:
                    pin = hs.enter_context(tc.tile_pool(name=f"pin_{rep_i}", bufs=1))
                    pqk = hs.enter_context(tc.tile_pool(name=f"pqk_{rep_i}", bufs=1))
                    phe = hs.enter_context(tc.tile_pool(name=f"phe_{rep_i}", bufs=4))
                    pho = hs.enter_context(tc.tile_pool(name=f"pho_{rep_i}", bufs=2))

                    # ---- input + weight DMAs, proj-critical first ----
                    w8 = {}
                    for nm, dram, eng in (("q", wq8_d, nc.sync),
                                          ("k", wk8_d, nc.scalar)):
                        wt = pin.tile([128, DK, D], FP8, tag=f"w8{nm}",
                                      name=f"w8{nm}")
                        eng.dma_start(out=wt[:], in_=dram[:, :, :])
                        w8[nm] = wt
                    q8t = pin.tile([128, DK, TP], FP8, tag="q8", name="q8t")
                    nc.sync.dma_start(out=q8t[:], in_=q8_d[:, :, :])
                    k8t = pin.tile([128, DK, TP], FP8, tag="k8", name="k8t")
                    nc.scalar.dma_start(out=k8t[:], in_=k8_d[:, :, :])
                    v8t = pin.tile([128, DK, 4 * BSP], FP8, tag="v8",
                                   name="v8t")
                    nc.gpsimd.dma_start(out=v8t[:], in_=v8_d[:, :, :])
                    for nm, dram, eng in (("v", wv8_d, nc.gpsimd),
                                          ("o", wo8_d, nc.sync)):
                        wt = pin.tile([128, DK, D], FP8, tag=f"w8{nm}",
                                      name=f"w8{nm}")
                        eng.dma_start(out=wt[:], in_=dram[:, :, :])
                        w8[nm] = wt

                    def load_q32(half):
                        # per-half residual stream; tags shared across halves
                        ts_ = []
                        for k in range(DK):
                            t_ = pqk.tile([128, TH], FP32R, tag=f"q32_{k}",
                                          name=f"q32_{half}{k}")
                            nc.gpsimd.dma_start(
                                out=t_[:],
                                in_=q32_d[k * 128:(k + 1) * 128,
                                          half * TH:half * TH + TH])
                            ts_.append(t_)
                        return ts_

                    q32h = {0: load_q32(0)}

                    # bv replicated to 128 rows, carrying S_AV*S_WV; folded
                    # into vh so softmax's row-sum applies it to ctx exactly
                    bvrow = rows.tile([1, D], FP32R, tag="bvrow", bufs=1)
                    nc.gpsimd.dma_start(
                        out=bvrow[:], in_=bv_d.rearrange("(a b) -> a b", a=1))
                    rowsc = const.tile([1, 128], FP32R, tag="rowsc")
                    with nc.allow_low_precision(reason="fp32r operand"):
                        nc.vector.tensor_scalar_mul(rowsc[:], ones_row_r[:],
                                                    S_AV * S_WV)
                    bvrep = pin.tile([128, D], FP32, tag="bvrep", name="bvrep")
                    for ni in range(2):
                        pbv = pb(128, 384)
                        nc.tensor.matmul(pbv[:], rowsc[:],
                                         bvrow[:, ni * 384:(ni + 1) * 384],
                                         start=True, stop=True)
                        nc.vector.tensor_copy(bvrep[:, ni * 384:(ni + 1) * 384],
                                              pbv[:])

                    # ---- per-half emission helpers ----
                    qh_t = [pqk.tile([128, T], BF16, tag=f"qh{k}",
                                     name=f"qh{k}") for k in range(DK)]
                    kh_t = [pqk.tile([128, T], BF16, tag=f"kh{k}",
                                     name=f"kh{k}") for k in range(DK)]

                    def emit_proj(which, half, mi):
                        src8, dst, bcol = ((q8t, qh_t, bq_col)
                                           if which == "q"
                                           else (k8t, kh_t, bk_col))
                        wt = w8[which]
                        n8 = half * THP
                        nt = half * TH
                        ps = pa(128, TH)
                        for kp in range(DK // 2):
                            nc.tensor.matmul(
                                ps[:],
                                wt[:, 2 * kp:2 * kp + 2,
                                   mi * 128:(mi + 1) * 128],
                                src8[:, 2 * kp:2 * kp + 2, n8:n8 + TH],
                                start=(kp == 0), stop=(kp == DK // 2 - 1),
                                perf_mode=DR)
                        with nc.allow_low_precision(reason="bf16 qh/kh"):
                            nc.scalar.activation(dst[mi][:, nt:nt + TH],
                                                 ps[:], AF.Identity,
                                                 bias=bcol[:, mi:mi + 1],
                                                 scale=1.0 / (S_A * S_W))

                    vh = {}

                    def emit_vh(half, bl, ci):
                        c0, cl = BK[ci]
                        vt_ = pqk.tile([128, H, DH + 1], FP32R,
                                       tag=f"vh{half}{bl}{ci}",
                                       name=f"vh{half}{bl}{ci}")
                        nc.gpsimd.dma_start(out=vt_[:cl, :, DH:DH + 1],
                                            in_=aux1_d[:cl, 0:H])
                        voff = (half * 2 + bl) * BSP + c0
                        for ni in range(2):
                            ps = pa(128, 384)
                            for kp in range(DK // 2):
                                nc.tensor.matmul(
                                    ps[:cl, :],
                                    v8t[:, 2 * kp:2 * kp + 2, voff:voff + cl],
                                    w8["v"][:, 2 * kp:2 * kp + 2,
                                            ni * 384:(ni + 1) * 384],
                                    start=(kp == 0), stop=(kp == DK // 2 - 1),
                                    perf_mode=DR)
                            nc.vector.tensor_add(
                                vt_[:cl, ni * 6:(ni + 1) * 6, 0:DH],
                                ps[:cl, :].rearrange("p (h d) -> p h d", d=DH),
                                bvrep[:cl, ni * 384:(ni + 1) * 384].rearrange(
                                    "p (h d) -> p h d", d=DH))
                        vh[(half, bl, ci)] = vt_

                    # ---- heads: 3-stage software pipeline ----
                    cx8 = pqk.tile([128, DK, TP], FP8, tag="cx8", name="cx8")

                    def emit_scores(half, hh):
                        h0tok = half * TH
                        dm, ro = divmod(hh * DH, 128)
                        exps = []
                        for bl in range(2):
                            for ci, (c0, cl) in enumerate(BK):
                                ps = pa(128, TH)
                                nc.tensor.matmul(
                                    ps[:cl, :],
                                    kh_t[dm][ro:ro + DH,
                                             h0tok + bl * S + c0:
                                             h0tok + bl * S + c0 + cl],
                                    qh_t[dm][ro:ro + DH, h0tok:h0tok + TH],
                                    start=True, stop=True)
                                ex = phe.tile([128, TH], FP32R, tag="exp",
                                              bufs=10)
                                nc.scalar.activation(ex[:cl, :], ps[:cl, :],
                                                     AF.Exp, scale=0.125)
                                exps.append((ex, cl))
                        return exps

                    def emit_pctx(half, hh, exps):
                        pcs = []
                        for bl in range(2):
                            pctx = pb(DH + 1, TH)
                            for ci in range(2):
                                ex, cl = exps[bl * 2 + ci]
                                nc.tensor.matmul(
                                    pctx[:], vh[(half, bl, ci)][:cl, hh, :],
                                    ex[:cl, :], start=(ci == 0),
                                    stop=(ci == 1))
                            pcS = phe.tile([DH + 1, TH], FP32R, tag="pcS",
                                           bufs=7)
                            with nc.allow_low_precision(reason="fp32r evict"):
                                nc.scalar.copy(pcS[:], pctx[:])
                            pcs.append(pcS)
                        return pcs

                    def emit_div(half, hh, pcs):
                        h8off = half * THP
                        dm, ro = divmod(hh * DH, 128)
                        srec = rows.tile([65, TH], FP32R, tag="srec", bufs=2)
                        with nc.allow_low_precision(reason="fp32r rep"):
                            nc.vector.reciprocal(srec[64:65, 0:S],
                                                 pcs[0][64:65, 0:S])
                            nc.vector.reciprocal(srec[64:65, S:TH],
                                                 pcs[1][64:65, S:TH])
                        prep = pb(DH, TH)
                        nc.tensor.matmul(prep[:], ones64r[64:65, 0:DH],
                                         srec[64:65, :], start=True, stop=True)
                        prs = phe.tile([64, TH], FP32, tag="prs", bufs=3)
                        nc.scalar.copy(prs[:], prep[:])
                        for bl in range(2):
                            bc = bl * S
                            with nc.allow_low_precision(reason="fp8 ctx"):
                                if ro == 0:
                                    nc.vector.tensor_mul(
                                        cx8[0:DH, dm,
                                            h8off + bc:h8off + bc + S],
                                        pcs[bl][0:DH, bc:bc + S],
                                        prs[:, bc:bc + S])
                                else:
                                    co = pho.tile([64, S], FP8, tag="cxodd")
                                    nc.vector.tensor_mul(
                                        co[:], pcs[bl][0:DH, bc:bc + S],
                                        prs[:, bc:bc + S])
                                    nc.gpsimd.dma_start(
                                        out=cx8[64:128, dm,
                                                h8off + bc:h8off + bc + S],
                                        in_=co[:])

                    r1h = {}

                    def emit_wo_mi(half, mi):
                        n8 = half * THP
                        if half not in r1h:
                            r1h[half] = [
                                pqk.tile([128, TH], FP32R, tag=f"r1_{j}",
                                         name=f"r1_{half}{j}")
                                for j in range(DK)]
                        ps = pa(128, TH)
                        for kp in range(DK // 2):
                            nc.tensor.matmul(
                                ps[:],
                                w8["o"][:, 2 * kp:2 * kp + 2,
                                        mi * 128:(mi + 1) * 128],
                                cx8[:, 2 * kp:2 * kp + 2, n8:n8 + TH],
                                start=(kp == 0), stop=(kp == DK // 2 - 1),
                                perf_mode=DR)
                        wo_t = tmp.tile([128, TH], FP32, tag="wo_t")
                        nc.scalar.activation(
                            wo_t[:], ps[:], AF.Identity,
                            bias=bo_col[:, mi:mi + 1],
                            scale=1.0 / (S_AV * S_WV * S_W))
                        nc.vector.tensor_add(
                            r1h[half][mi][:], wo_t[:], q32h[half][mi][:])

                    def emit_ln1(half):
                        if half == 0:
                            q32h[1] = load_q32(1)
                        layer_norm(r1h[half], l1g_col, l1b_col, x_t,
                                   half * TH, [(0, TH)],
                                   out8=x8, g8_col=l1g8_col, b8_col=l1b8_col)

                    # ---- emission: half 0 attention with half-1 prep
                    # interleaved as PE gap filler ----
                    for mi in range(DK):
                        emit_proj("q", 0, mi)
                    for mi in range(DK):
                        emit_proj("k", 0, mi)
                    for bl in range(2):
                        for ci in range(2):
                            emit_vh(0, bl, ci)

                    def run_heads(half, side=None):
                        st = {}
                        pc = {}
                        side = list(side or [])
                        for i in range(H):
                            st[i] = emit_scores(half, i)
                            if side:
                                side.pop(0)()
                            if i >= 1:
                                pc[i - 1] = emit_pctx(half, i - 1,
                                                      st.pop(i - 1))
                            if i >= 3:
                                emit_div(half, i - 3, pc.pop(i - 3))
                        pc[H - 1] = emit_pctx(half, H - 1, st.pop(H - 1))
                        for i in (H - 3, H - 2, H - 1):
                            if side:
                                side.pop(0)()
                            emit_div(half, i, pc.pop(i))
                        while side:
                            side.pop(0)()

                    def emit_gates(half):
                        n0 = half * TH
                        nl = TH
                        pg = pb(8, TH)
                        for k in range(DK):
                            nc.tensor.matmul(pg[:], wgs[:, k, :],
                                             x_t[k][:, n0:n0 + nl],
                                             start=(k == 0),
                                             stop=(k == DK - 1))
                        nc.scalar.activation(gexp[:, n0:n0 + nl], pg[:],
                                             AF.Exp, bias=bg_col[:])
                        pgs = pb(1, TH)
                        nc.tensor.matmul(pgs[:], ones8_col[:],
                                         gexp[:, n0:n0 + nl],
                                         start=True, stop=True)
                        grec = rows.tile([1, TH], FP32R, tag="grec", bufs=1)
                        with nc.allow_low_precision(reason="fp32r operand"):
                            nc.vector.reciprocal(grec[:], pgs[:])
                        pgr = pb(8, TH)
                        nc.tensor.matmul(pgr[:], ones_row8_r[:], grec[:],
                                         start=True, stop=True)
                        nc.vector.tensor_mul(gate[:, n0:n0 + nl],
                                             gexp[:, n0:n0 + nl], pgr[:])

                    side1 = ([lambda mi=mi: emit_proj("q", 1, mi)
                              for mi in range(DK)] +
                             [lambda mi=mi: emit_proj("k", 1, mi)
                              for mi in range(DK)] +
                             [lambda bl=bl, ci=ci: emit_vh(1, bl, ci)
                              for bl in range(2) for ci in range(2)])
                    run_heads(0, side1)
                    # half-0 Wo/LN1/gates ride inside half-1's head pipeline
                    side2 = ([lambda mi=mi: emit_wo_mi(0, mi)
                              for mi in range(DK)] +
                             [lambda: emit_ln1(0), lambda: emit_gates(0)])
                    run_heads(1, side2)
                    for mi in range(DK):
                        emit_wo_mi(1, mi)
                    emit_ln1(1)
                    emit_gates(1)

            # ================= moe accumulators =================
            if rep_i == 0:
                # opened after the attention pools close, so it reuses
                # their SBUF; lives through LN2 + output
                moepool = top.enter_context(
                    tc.tile_pool(name="moepool", bufs=1))
                moe = [moepool.tile([128, T], FP32, tag=f"moe{k}",
                                    name=f"moe{k}") for k in range(DK)]

            # moe_acc init = gates^T @ b2   (lhsT = b2 chunks [8, 128])
            for mi in range(DK):
                for (n0, nl) in NCH:
                    pbi = pa(128, TH)
                    nc.tensor.matmul(pbi[:], b2s[:, mi * 128:(mi + 1) * 128],
                                     gate[:, n0:n0 + nl], start=True, stop=True)
                    nc.scalar.copy(moe[mi][:, n0:n0 + nl], pbi[:])

            # ================= MoE experts (fp8 DoubleRow) =================
            FK2 = FK // 2  # 12 f-subtile pairs
            with ExitStack() as ms:
              if not skip_moe:
                  pmh = ms.enter_context(tc.tile_pool(name=f"pmh_{rep_i}", bufs=14))
                  pmw = ms.enter_context(tc.tile_pool(name=f"pmw_{rep_i}", bufs=2))
                  pmt = ms.enter_context(tc.tile_pool(name=f"pmt_{rep_i}", bufs=2))
                  pfo = ms.enter_context(tc.tile_pool(name=f"pfo_{rep_i}", bufs=2))

                  def emit_tail(half):
                      n0 = half * TH
                      for mi in range(DK):
                          nc.vector.tensor_add(x_t[mi][:, n0:n0 + TH],
                                               x_t[mi][:, n0:n0 + TH],
                                               moe[mi][:, n0:n0 + TH])
                      layer_norm(x_t, l2g_col, l2b_col, moe, 0, [(n0, TH)])
                      for (t0, tl) in OTT:
                          ot = pfo.tile([128, D], FP32, tag="otok")
                          for k in range(DK):
                              ps = pa(128, 128)
                              nc.tensor.transpose(
                                  ps[:tl, :], moe[k][:, n0 + t0:n0 + t0 + tl],
                                  ident[:, :])
                              nc.vector.tensor_copy(
                                  ot[:tl, k * 128:(k + 1) * 128], ps[:tl, :])
                          nc.sync.dma_start(out=out_d[n0 + t0:n0 + t0 + tl, :],
                                            in_=ot[:tl, :])
                  for e in range(E):
                      braw = rows.tile([FK, 128], FP32, tag="rawb1")
                      nc.sync.dma_start(
                          out=braw[:], in_=b1_d[e].rearrange("(a b) -> a b", b=128))
                      pbv = pb(128, FK)
                      nc.tensor.transpose(pbv[:], braw[:], ident[:FK, :FK])
                      b1c = rows.tile([128, FK], FP32, tag="b1col")
                      nc.vector.tensor_copy(b1c[:], pbv[:])

                      # gate row broadcast to 128 partitions, evicted to SBUF
                      # (sel8 carries 1/S_W2, descaling the y-phase matmul)
                      grep = pmt.tile([128, T], FP32, tag="gerep")
                      for (n0, nl) in NCH:
                          pge = pb(128, TH)
                          nc.tensor.matmul(pge[:],
                                           sel8[:, e * 128:(e + 1) * 128],
                                           gate[:, n0:n0 + nl],
                                           start=True, stop=True)
                          nc.vector.tensor_copy(grep[:, n0:n0 + nl], pge[:])

                      # whole-expert fp8 weights, one DMA each
                      w1t = pmw.tile([128, DK, F], FP8, tag="w1", name="w1t")
                      nc.sync.dma_start(out=w1t[:], in_=w1_d[e])
                      w2t = pmw.tile([128, FK2, 2, D], FP8, tag="w2",
                                     name="w2t")
                      nc.sync.dma_start(out=w2t[:], in_=w2_d[e])

                      # ---- h + y phases; the last expert runs each token
                      # half to completion so half 0's LN2/output overlaps
                      # half 1's matmuls ----
                      ni_groups = ([(0, 1)] if e < E - 1 else [(0,), (1,)])
                      for nig in ni_groups:
                          h8s = []
                          for f2 in range(FK2):
                              h8 = pmh.tile([128, 2, TP], FP8, tag="h",
                                            name="h8")
                              for jj in range(2):
                                  fm = 2 * f2 + jj
                                  phs = {0: pa, 1: pb}
                                  for ni in nig:
                                      ph_ = phs[ni] = phs[ni](128, TH)
                                      n8 = NCH8[ni][0]
                                      for kp in range(DK // 2):
                                          nc.tensor.matmul(
                                              ph_[:],
                                              w1t[:, 2 * kp:2 * kp + 2,
                                                  fm * 128:(fm + 1) * 128],
                                              x8[:, 2 * kp:2 * kp + 2,
                                                 n8:n8 + TH],
                                              start=(kp == 0),
                                              stop=(kp == DK // 2 - 1),
                                              perf_mode=DR)
                                  with nc.allow_low_precision(reason="fp8 h"):
                                      for ni in nig:
                                          n8 = NCH8[ni][0]
                                          nc.scalar.activation(
                                              h8[:, jj, n8:n8 + TH],
                                              phs[ni][:], AF.Gelu,
                                              bias=b1c[:, fm:fm + 1],
                                              scale=1.0 / (S_X * S_W1))
                              h8s.append(h8)

                          for dg in range(3):
                              pys = {}
                              for ni in nig:
                                  pys[(0, ni)] = pa(128, TH)
                                  pys[(1, ni)] = pb(128, TH)
                              for f2 in range(FK2):
                                  for j in range(2):
                                      w2s = w2t[:, f2, :,
                                                dg * 256 + j * 128:
                                                dg * 256 + (j + 1) * 128]
                                      for ni in nig:
                                          n0, nl = NCH8[ni]
                                          nc.tensor.matmul(
                                              pys[(j, ni)][:], w2s,
                                              h8s[f2][:, :, n0:n0 + nl],
                                              start=(f2 == 0),
                                              stop=(f2 == FK2 - 1),
                                              perf_mode=DR)
                              for j in range(2):
                                  mi = dg * 2 + j
                                  for ni in nig:
                                      n0, nl = NCH[ni]
                                      ty = pmt.tile([128, TH], FP32, tag="ty")
                                      nc.vector.tensor_mul(ty[:],
                                                           pys[(j, ni)][:],
                                                           grep[:, n0:n0 + nl])
                                      nc.vector.tensor_add(
                                          moe[mi][:, n0:n0 + nl],
                                          moe[mi][:, n0:n0 + nl], ty[:])
                          if e == E - 1:
                              emit_tail(nig[0])



    nc.compile()
    return nc


def _get_nc(**flags):
    key = tuple(sorted(flags.items()))
    if key not in _CACHE:
        _CACHE[key] = _build(**flags)
    return _CACHE[key]


def _q8(x, scale):
    import ml_dtypes
    return np.clip(x * scale, -224.0, 224.0).astype(ml_dtypes.float8_e4m3)


def _pack_w8(w, scale):
    # [D, D] -> [128, DK, D] with [p, kb, dout] = q8(w[kb*128+p, dout]*scale)
    return np.ascontiguousarray(
        _q8(w, scale).reshape(DK, 128, D).transpose(1, 0, 2))


def _pack_act8(a, scale, nseg, seg, stride):
    # [nseg*seg, D] tokens -> [128, DK, nseg*stride] feature-major fp8,
    # each token segment padded to `stride`
    q = _q8(a, scale).reshape(nseg, seg, DK, 128).transpose(3, 2, 0, 1)
    out = np.zeros((128, DK, nseg, stride), q.dtype)
    out[:, :, :, :seg] = q
    return np.ascontiguousarray(out.reshape(128, DK, nseg * stride))


def run(inputs, _flags=None, **spmd_kwargs):
    nc = _get_nc(**(_flags or {}))
    inp = {k: np.ascontiguousarray(np.asarray(v, dtype=np.float32))
           for k, v in inputs.items()}
    shared = {k: v for k, v in inp.items()
              if k not in ("q", "k", "v", "W1", "W2",
                           "Wq", "Wk", "Wv", "Wo")}
    # fp8-packed expert weights:
    #   W1q[e, p, kb, f] = q8(W1[e, kb*128+p, f] * S_W1)
    #   W2q[e, p, f2, j, d] = q8(W2[e, (2*f2+j)*128+p, d] * S_W2)
    shared["W1q"] = np.ascontiguousarray(
        _q8(inp["W1"], S_W1).reshape(E, DK, 128, F).transpose(0, 2, 1, 3))
    shared["W2q"] = np.ascontiguousarray(
        _q8(inp["W2"], S_W2).reshape(E, F // 256, 2, 128, D).transpose(
            0, 3, 1, 2, 4))
    shared["Wq8"] = _pack_w8(inp["Wq"], S_W)
    shared["Wk8"] = _pack_w8(inp["Wk"], S_W)
    shared["Wv8"] = _pack_w8(inp["Wv"], S_WV)
    shared["Wo8"] = _pack_w8(inp["Wo"], S_W)
    sel = np.zeros((E, E * 128), dtype=np.float32)
    for e in range(E):
        sel[e, e * 128:(e + 1) * 128] = 1.0 / S_W2
    shared["sel8"] = sel
    shared["aux_ones"] = np.ones((128, 128), dtype=np.float32)
    a64 = np.zeros((65, 128), dtype=np.float32)
    a64[64, :] = 1.0
    shared["aux_ones64"] = a64
    shared["aux_eps"] = np.full((1, 1), EPS, dtype=np.float32)
    in_maps = []
    for c in range(NCORES):
        m = dict(shared)
        for name in ("q", "k", "v"):
            a = np.ascontiguousarray(
                inp[name][c * BPC:(c + 1) * BPC].reshape(T, D))
            if name == "v":
                m["v8"] = _pack_act8(a, S_AV, 4, S, BSP)
            else:
                m[name + "8"] = _pack_act8(a, S_A, 2, TH, THP)
            if name == "q":
                m["q32"] = np.ascontiguousarray(a.T)
        in_maps.append(m)
    res = run_bass_kernel_spmd(nc, in_maps, core_ids=list(range(NCORES)),
                               **spmd_kwargs)
    out = np.stack([r["out"] for r in res.results])  # [8, T, D]
    return out.reshape(B, S, D), res


def kernel(**inputs):
    out, _ = run(inputs)
    return out

